# revision 3
# baseline (speedup 1.0000x reference)
"""CPC loss (nn_CPCLossV2) Trainium2 Bass kernel — minimal-wire version.

Problem: n=4096 groups x k=4 rows of h=256 embeddings.
  hist_x[g]  = rows 4g..4g+2 concat -> [n, 768]
  predicts   = hist_x @ W + b       -> [n, 256]
  logits[g]  = [predicts[g].emb[4g+3], predicts[g].emb[neg_idx[g, 0..63]]]
  loss       = mean_g(logsumexp(logits_g) - logits_g[0])

Host->device transfer over the axon tunnel plus a fixed ~78ms/call
dispatch round trip dominate the wall clock (device compute is ~2ms), so
the kernel minimizes wire bytes AND input-tensor count:

  * ALL per-core inputs ship as ONE packed uint8 tensor (647KB/core,
    ~6ms/array overhead saved vs 7 tensors); device-side views unpack it
    via AP bitcast.
  * embeddings ship SHARDED in fp8 e4m3 (2048 rows/core, transposed) and
    are AllGathered on-device over the device fabric; W ships bf16
    sharded by output column (32/core), AllGathered too.
  * ALL 16384 logits per group are computed on-device by PE matmul
    (l^T[r, g] = sum_h embT[h, r] * predsT[h, g]) into a bf16 tile
    LT[hi, g, lo] with r = 128*lo + hi.
  * the 65 needed logits per group (positive at j=0, 64 negatives) are
    selected on-device with a two-level one-hot gather (no GPSIMD custom
    ucode on this bedrock image, so no ap_gather/dma_gather): a
    per-group one-hot matmul picks partition hi, then an is_equal mask +
    reduce picks lo.  Host ships only uint8 hi/lo index planes.
  * fp8 quantization of the table perturbs the loss by ~7e-4 relative
    (validated against the fp32 reference in sim and on HW); the
    correctness gate is 2e-2.
  * per-group logsumexp epilogue identical to the reference; per-core
    partial sums returned as [128, 1] and combined on host.
"""

import os
from contextlib import ExitStack

import numpy as np
import ml_dtypes

N = 4096          # groups
K = 4             # rows per group
H = 256           # embedding dim
M = 64            # negatives per group
J = M + 1         # selections per group (j=0 is the positive)
NCORES = 8
S = N // NCORES   # 512 groups per core
ROWS = S * K      # 2048 local rows
RT = N * K        # 16384 total rows
BANDS = S // 128  # 4 bands of 128 groups
GB = 7            # hiRep psum batch: GB*J = 455 <= 512 fp32 psum cols
WSH = H // NCORES  # 32 W columns per core

AGE_W = (K - 1) * H * WSH     # 24576 bf16 elems in the W shard

# packed single-input layout, in 2048-byte rows:
#   [0:256)    embT shard, fp8, [256, 2048]
#   [256:280)  W shard, bf16 row-major [768, 32] = 49152 B
#   [280]      bvec f32 [256] in bytes 0:1024
#   [281:298)  idxhi u8, 33280 B used
#   [298:315)  idxloT u8 [65, 512], 33280 B used
#   [315]      ones bf16 [1, 128] bytes 0:256, iotaf4 bf16 [1, 512] bytes 256:1280
PKROWS = 316
PK_W, PK_B, PK_HI, PK_LO, PK_C = 256, 280, 281, 298, 315

_CACHE = {}


# --------------------------------------------------------------------------
# device program
# --------------------------------------------------------------------------

def build_nc(debug=False):
    import concourse.bass as bass
    import concourse.tile as tile
    from concourse import bacc, masks, mybir

    f32 = mybir.dt.float32
    bf16 = mybir.dt.bfloat16
    Alu = mybir.AluOpType
    Act = mybir.ActivationFunctionType
    Ax = mybir.AxisListType

    nc = bacc.Bacc(
        "TRN2", target_bir_lowering=False, debug=debug, num_devices=NCORES
    )

    f8 = mybir.dt.float8e4
    u8 = mybir.dt.uint8
    pkh = nc.dram_tensor("pk", [PKROWS, ROWS], u8, kind="ExternalInput")
    pk = pkh.ap()
    lossp = nc.dram_tensor("loss_part", [128, 1], f32, kind="ExternalOutput").ap()

    embT_sh = pk[0:PK_W, :].bitcast(f8)                       # [256, 2048]
    Wsh_flat = pk[PK_W:PK_B, :].bitcast(bf16)                 # [24, 1024]
    bvec_row = pk[PK_B : PK_B + 1, 0 : H * 4].bitcast(f32)    # [1, 256]
    idxloT = bass.AP(pkh, PK_LO * ROWS, [[S, J], [1, S]])     # [65, 512] u8
    ones_in = pk[PK_C : PK_C + 1, 0:256].bitcast(bf16)        # [1, 128]
    iotaf4 = pk[PK_C : PK_C + 1, 256:1280].bitcast(bf16)      # [1, 512]

    def idxhi_band(B):
        off = PK_HI * ROWS + B * 128 * J
        return bass.AP(pkh, off, [[128 * J, 1], [1, 128 * J]])

    with tile.TileContext(nc) as tc, ExitStack() as ctx:
        dram = ctx.enter_context(tc.tile_pool(name="dram", bufs=1, space="DRAM"))
        cpool = ctx.enter_context(tc.tile_pool(name="const", bufs=1))
        oh1pool = ctx.enter_context(tc.tile_pool(name="oh1", bufs=2))
        zpool = ctx.enter_context(tc.tile_pool(name="zp", bufs=2))
        pslog = ctx.enter_context(tc.tile_pool(name="pslog", bufs=2, space="PSUM"))
        psh = ctx.enter_context(tc.tile_pool(name="psh", bufs=2, space="PSUM"))
        psT = ctx.enter_context(tc.tile_pool(name="psT", bufs=2, space="PSUM"))
        psz = ctx.enter_context(tc.tile_pool(name="psz", bufs=2, space="PSUM"))

        # ---- all-gather emb + W shards over the device fabric ---------------
        agin_e = dram.tile([H, ROWS], f8, tag="agin_e")
        agout_e = dram.tile([NCORES, H, ROWS], f8, tag="agout_e")
        agin_w = dram.tile([AGE_W // ROWS, ROWS], bf16, tag="agin_w")
        agout_w = dram.tile([NCORES, AGE_W // ROWS, ROWS], bf16, tag="agout_w")
        nc.sync.dma_start(out=agin_e[:], in_=embT_sh)
        nc.sync.dma_start(
            out=agin_w[:],
            in_=Wsh_flat.rearrange("(a c) y -> a (c y)", c=2),
        )
        nc.gpsimd.collective_compute(
            "AllGather",
            Alu.bypass,
            replica_groups=[list(range(NCORES))],
            ins=[agin_e.opt()],
            outs=[agout_e.opt()],
        )
        nc.gpsimd.collective_compute(
            "AllGather",
            Alu.bypass,
            replica_groups=[list(range(NCORES))],
            ins=[agin_w.opt()],
            outs=[agout_w.opt()],
        )

        # ---- SBUF constant loads -------------------------------------------
        embT_loc = []
        for hc in range(2):
            t = cpool.tile([128, ROWS], f8, tag=f"embT_loc{hc}")
            nc.sync.dma_start(out=t[:], in_=embT_sh[128 * hc : 128 * (hc + 1), :])
            embT_loc.append(t)
        bias_sb = []
        for mc in range(2):
            t = cpool.tile([128, 1], f32, tag=f"bias{mc}")
            nc.sync.dma_start(
                out=t[:], in_=bvec_row[:, 128 * mc : 128 * (mc + 1)]
            )
            bias_sb.append(t)
        ihpool = ctx.enter_context(tc.tile_pool(name="ihp", bufs=2))
        idxloT_u8 = cpool.tile([J, S], u8, tag="idxloT_u8")
        nc.sync.dma_start(out=idxloT_u8[:], in_=idxloT)
        idxloT_sb = cpool.tile([J, S], bf16, tag="idxloT_sb")
        nc.vector.tensor_copy(idxloT_sb[:], idxloT_u8[:])
        ones_sb = cpool.tile([1, 128], bf16, tag="ones_sb")
        nc.sync.dma_start(out=ones_sb[:], in_=ones_in)
        iotaf4_sb = cpool.tile([1, 512], bf16, tag="iotaf4_sb")
        nc.sync.dma_start(out=iotaf4_sb[:], in_=iotaf4)
        iotap_sb = cpool.tile([128, 512], bf16, tag="iotap_sb")
        nc.gpsimd.iota(
            iotap_sb[:], [[0, 512]], channel_multiplier=1,
            allow_small_or_imprecise_dtypes=True,
        )
        ident = cpool.tile([128, 128], f32, tag="ident")
        masks.make_identity(nc, ident[:])

        # gathered full table / W: wait on agout then spread into SBUF
        embT_full = []
        ag_emb = agout_e[:].rearrange("c h r -> h c r")
        for hc in range(2):
            t = cpool.tile([128, RT], f8, tag=f"embT_full{hc}")
            nc.sync.dma_start(
                out=t[:].rearrange("p (c r) -> p c r", c=NCORES),
                in_=ag_emb[128 * hc : 128 * (hc + 1)],
            )
            embT_full.append(t)
        ag_w = agout_w[:].rearrange(
            "c a (b w) -> (a b) c w", w=WSH
        )
        W_full = []
        for kc in range(6):
            t = cpool.tile([128, H], bf16, tag=f"W_full{kc}")
            nc.sync.dma_start(
                out=t[:].rearrange("p (c w) -> p c w", c=NCORES),
                in_=ag_w[128 * kc : 128 * (kc + 1)],
            )
            W_full.append(t)

        # ---- predsT = (hist_x @ W + b)^T : [h, g] in bf16 -------------------
        predsT = []
        for mc in range(2):
            pt = psz.tile([128, S], f32, tag="ps_misc")
            for j in range(K - 1):
                for hc in range(2):
                    kc = 2 * j + hc
                    rhs = embT_loc[hc][:].rearrange("p (g j) -> p j g", j=K)[:, j, :]
                    nc.tensor.matmul(
                        pt[:],
                        lhsT=W_full[kc][:, 128 * mc : 128 * (mc + 1)],
                        rhs=rhs,
                        start=(kc == 0),
                        stop=(kc == 5),
                    )
            t = cpool.tile([128, S], bf16, tag=f"predsT{mc}")
            nc.vector.tensor_scalar_add(t[:], pt[:], bias_sb[mc][:])
            predsT.append(t)

        # ---- iota along lo, replicated on 65 partitions / 4 group slots -----
        ps_i = psz.tile([J, 512], f32, tag="ps_misc")
        nc.tensor.matmul(
            ps_i[:], lhsT=ones_sb[:, 0:J], rhs=iotaf4_sb[:], start=True, stop=True
        )
        iota65_4 = cpool.tile([J, 4, 128], f32, tag="iota65_4")
        nc.vector.tensor_copy(iota65_4[:], ps_i[:].rearrange("p (a b) -> p a b", a=4))

        # ---- per-band: logits, two-level gather, logsumexp ------------------
        LT = cpool.tile([128, 128, 128], bf16, tag="LT")  # [hi, g, lo]
        mx_b = cpool.tile([128, BANDS], f32, tag="mx_b")
        sume_b = cpool.tile([128, BANDS], f32, tag="sume_b")
        pos_b = cpool.tile([128, BANDS], f32, tag="pos_b")
        scr = cpool.tile([128, J], f32, tag="scr")

        for B in range(BANDS):
            # all 16384 logits for the band's 128 groups
            for lo in range(128):
                pl = pslog.tile([128, 128], f32, tag="ps_log")
                for mc in range(2):
                    nc.tensor.matmul(
                        pl[:],
                        lhsT=embT_full[mc][:, 128 * lo : 128 * (lo + 1)],
                        rhs=predsT[mc][:, 128 * B : 128 * (B + 1)],
                        start=(mc == 0),
                        stop=(mc == 1),
                    )
                nc.vector.tensor_copy(LT[:, :, lo], pl[:])

            # one-hot over hi for every (group, j) of the band
            ihb_u8 = ihpool.tile([1, 128 * J], u8, tag="ihb_u8")
            nc.sync.dma_start(out=ihb_u8[:], in_=idxhi_band(B))
            ihb = ihpool.tile([1, 128 * J], bf16, tag="ihb")
            nc.vector.tensor_copy(ihb[:], ihb_u8[:])
            OH1 = oh1pool.tile([128, 128 * J], bf16, tag="OH1")
            for gb in range(0, 128, GB):
                nb = min(GB, 128 - gb)
                ph = psh.tile([128, GB * J], f32, tag="ps_hi")
                nc.tensor.matmul(
                    ph[:, 0 : nb * J],
                    lhsT=ones_sb[:],
                    rhs=ihb[:, gb * J : (gb + nb) * J],
                    start=True,
                    stop=True,
                )
                nc.vector.tensor_tensor(
                    OH1[:, gb * J : (gb + nb) * J],
                    ph[:, 0 : nb * J],
                    iotap_sb[:, 0 : nb * J],
                    op=Alu.is_equal,
                )

            # T[j, lo] = LT[hi_j, g, lo] via one-hot matmul; pick lo by mask
            Z = zpool.tile([J, 128], f32, tag="Z")
            for q4 in range(0, 128, 4):
                pT = psT.tile([J, 4, 128], f32, tag="ps_T")
                for u in range(4):
                    g = q4 + u
                    nc.tensor.matmul(
                        pT[:, u, :],
                        lhsT=OH1[:, g * J : (g + 1) * J],
                        rhs=LT[:, g, :],
                        start=True,
                        stop=True,
                    )
                oh2 = zpool.tile([J, 4, 128], bf16, tag="oh2")
                locols = idxloT_sb[:, B * 128 + q4 : B * 128 + q4 + 4]
                nc.vector.tensor_tensor(
                    oh2[:],
                    iota65_4[:],
                    locols.unsqueeze(2).broadcast_to([J, 4, 128]),
                    op=Alu.is_equal,
                )
                prod = zpool.tile([J, 4, 128], f32, tag="prod")
                nc.vector.tensor_tensor(prod[:], pT[:], oh2[:], op=Alu.mult)
                nc.vector.tensor_reduce(
                    Z[:, q4 : q4 + 4], prod[:], axis=Ax.X, op=Alu.add
                )

            # transpose Z -> [g, j]; logsumexp pieces
            pz = psz.tile([128, J], f32, tag="ps_misc")
            nc.tensor.transpose(pz[:], Z[:], ident[0:J, 0:J])
            nc.vector.tensor_reduce(
                mx_b[:, B : B + 1], pz[:], axis=Ax.X, op=Alu.max
            )
            negmx = zpool.tile([128, 1], f32, tag="negmx")
            nc.vector.tensor_scalar_mul(negmx[:], mx_b[:, B : B + 1], -1.0)
            nc.scalar.activation(
                scr[:],
                pz[:],
                Act.Exp,
                bias=negmx[:],
                accum_out=sume_b[:, B : B + 1],
            )
            nc.vector.tensor_copy(pos_b[:, B : B + 1], pz[:, 0:1])

        # ---- loss_pg = ln(sume) + mx - pos; partial sum out -----------------
        lse = cpool.tile([128, BANDS], f32, tag="lse")
        nc.scalar.activation(lse[:], sume_b[:], Act.Ln)
        nc.vector.tensor_tensor(lse[:], lse[:], mx_b[:], op=Alu.add)
        nc.vector.tensor_tensor(lse[:], lse[:], pos_b[:], op=Alu.subtract)
        lred = cpool.tile([128, 1], f32, tag="lred")
        nc.vector.tensor_reduce(lred[:], lse[:], axis=Ax.X, op=Alu.add)
        nc.sync.dma_start(out=lossp, in_=lred[:])

    nc.compile()
    return nc


# --------------------------------------------------------------------------
# host-side sharding
# --------------------------------------------------------------------------

def _neg_indices(target, perm, k, m):
    """neg_idx[g, j] = cand[g][perm[g, j]] exactly as the reference builds it."""
    n = target.shape[0] // k
    t64 = np.asarray(target)
    expected = np.repeat(np.arange(n, dtype=t64.dtype), k)
    p = np.asarray(perm)[:, :m].astype(np.int64)
    if np.array_equal(t64, expected):
        # cand[g][j] = j if j < k*g else j + k
        g = np.arange(n, dtype=np.int64)[:, None]
        return p + k * (p >= k * g)
    # generic (slow) fallback, matches jnp.where(..., size=k*(n-1), fill=0)
    group_t = t64[0::k]
    out = np.zeros((n, m), dtype=np.int64)
    order = np.arange(t64.shape[0], dtype=np.int64)
    for gi in range(n):
        cand = order[t64 != group_t[gi]]
        cand = np.pad(cand, (0, k * (n - 1) - cand.shape[0]))
        out[gi] = cand[p[gi]]
    return out


def _prep_inputs(embeddings, W, b, target, perm, k, m):
    bf16 = ml_dtypes.bfloat16
    f8 = ml_dtypes.float8_e4m3
    emb = np.asarray(embeddings, dtype=np.float32)
    Wf = np.asarray(W, dtype=np.float32).astype(bf16)
    bf = np.asarray(b, dtype=np.float32).reshape(H, 1)
    neg_idx = _neg_indices(target, perm, k, m)  # [N, M] global rows

    # selection table: j=0 positive row (4g+3), then the 64 negatives
    gidx = np.arange(N, dtype=np.int64)
    sel = np.empty((N, J), dtype=np.int64)
    sel[:, 0] = K * gidx + (K - 1)
    sel[:, 1:] = neg_idx
    hi = (sel % 128).astype(np.uint8)
    lo = (sel // 128).astype(np.uint8)

    ones = np.ones((1, 128), dtype=bf16)
    iotaf4 = np.tile(np.arange(128, dtype=np.float32), 4)[None, :].astype(bf16)

    in_maps = []
    for c in range(NCORES):
        sl = emb[ROWS * c : ROWS * (c + 1)]
        embT = np.ascontiguousarray(sl.T.astype(f8))
        hic = np.ascontiguousarray(hi[S * c : S * (c + 1)].reshape(S * J))
        loc = np.ascontiguousarray(lo[S * c : S * (c + 1)].T)
        pk = np.zeros((PKROWS, ROWS), dtype=np.uint8)
        pk[0:PK_W] = embT.view(np.uint8)
        wsh = np.ascontiguousarray(Wf[:, WSH * c : WSH * (c + 1)])
        pk[PK_W:PK_B] = wsh.view(np.uint8).reshape(PK_B - PK_W, ROWS)
        pk[PK_B, 0 : H * 4] = bf.astype(np.float32).view(np.uint8).reshape(-1)
        pk[PK_HI:PK_LO].reshape(-1)[0 : S * J] = hic
        pk[PK_LO:PK_C].reshape(-1)[0 : J * S] = loc.reshape(-1)
        pk[PK_C, 0:256] = ones.view(np.uint8).reshape(-1)
        pk[PK_C, 256:1280] = iotaf4.view(np.uint8).reshape(-1)
        in_maps.append({"pk": pk})
    return in_maps


def _run(nc, in_maps):
    """Execute the compiled module on the 8 cores.

    Same lowering as bass_utils.run_bass_kernel_spmd's axon path
    (bass2jax.run_bass_via_pjrt), but the jit-wrapped shard_map callable is
    built ONCE and cached: run_bass_kernel_spmd re-creates the closure per
    call, which forces a full jax retrace + XLA rebuild (~0.3s) on every
    invocation even though the NEFF itself is cached.  Falls back to
    run_bass_kernel_spmd if the fast path can't initialize.
    """
    if "runner" not in _CACHE:
        try:
            _CACHE["runner"] = _make_runner(nc)
        except Exception:
            _CACHE["runner"] = None
    runner = _CACHE["runner"]
    if runner is not None:
        return runner(in_maps)

    from concourse.bass_utils import run_bass_kernel_spmd

    res = run_bass_kernel_spmd(nc, in_maps, list(range(NCORES)))
    return [res.results[c] for c in range(NCORES)]


def _make_runner(nc):
    import jax
    from jax.sharding import Mesh, PartitionSpec
    try:
        from jax.experimental.shard_map import shard_map
    except ImportError:
        from jax import shard_map
    from concourse import mybir
    from concourse.bass2jax import (
        _bass_exec_p,
        install_neuronx_cc_hook,
        partition_id_tensor,
    )
    from concourse.bass_utils import axon_active

    if not axon_active():
        return None
    install_neuronx_cc_hook()
    assert nc.dbg_addr is None

    partition_name = (
        nc.partition_id_tensor.name if nc.partition_id_tensor else None
    )
    in_names, out_names, out_avals = [], [], []
    for alloc in nc.m.functions[0].allocations:
        if not isinstance(alloc, mybir.MemoryLocationSet):
            continue
        name = alloc.memorylocations[0].name
        if alloc.kind == "ExternalInput":
            if name != partition_name:
                in_names.append(name)
        elif alloc.kind == "ExternalOutput":
            out_names.append(name)
            out_avals.append(
                jax.core.ShapedArray(
                    tuple(alloc.tensor_shape), mybir.dt.np(alloc.dtype)
                )
            )
    n_params = len(in_names)
    n_outs = len(out_names)
    all_names = list(in_names) + list(out_names)
    if partition_name is not None:
        all_names.append(partition_name)

    def _body(*args):
        operands = list(args)
        if partition_name is not None:
            operands.append(partition_id_tensor())
        return tuple(
            _bass_exec_p.bind(
                *operands,
                out_avals=tuple(out_avals),
                in_names=tuple(all_names),
                out_names=tuple(out_names),
                lowering_input_output_aliases=(),
                sim_require_finite=True,
                sim_require_nnan=True,
                nc=nc,
            )
        )

    devices = jax.devices()[:NCORES]
    mesh = Mesh(np.asarray(devices), ("core",))
    donate = tuple(range(n_params, n_params + n_outs))
    sharded = jax.jit(
        shard_map(
            _body,
            mesh=mesh,
            in_specs=(PartitionSpec("core"),) * (n_params + n_outs),
            out_specs=(PartitionSpec("core"),) * n_outs,
            check_rep=False,
        ),
        donate_argnums=donate,
        keep_unused=True,
    )

    def runner(in_maps):
        concat_in = [
            np.concatenate([np.asarray(m[name]) for m in in_maps], axis=0)
            for name in in_names
        ]
        concat_zeros = [
            np.zeros((NCORES * a.shape[0], *a.shape[1:]), a.dtype)
            for a in out_avals
        ]
        out_arrs = sharded(*concat_in, *concat_zeros)
        return [
            {
                name: np.asarray(out_arrs[i]).reshape(
                    NCORES, *out_avals[i].shape
                )[c]
                for i, name in enumerate(out_names)
            }
            for c in range(NCORES)
        ]

    return runner


def kernel(embeddings, W, b, target, perm, k_pos_samples, m_neg_samples):
    k = int(k_pos_samples)
    m = min(int(m_neg_samples), k * (N - 1))
    assert k == K and m == M and embeddings.shape == (N * K, H)

    if "nc" not in _CACHE:
        _CACHE["nc"] = build_nc(debug=False)
    nc = _CACHE["nc"]

    in_maps = _prep_inputs(embeddings, W, b, target, perm, k, m)

    results = _run(nc, in_maps)
    total = 0.0
    for c in range(NCORES):
        total += float(np.sum(results[c]["loss_part"].astype(np.float64)))
    return np.float32(total / N)
